# revision 36
# baseline (speedup 1.0000x reference)
"""Trainium2 Bass kernel for channel-attention:
    scores[b,q,k] = sum_{h,w} Q[b,h,w,q] * K[b,h,w,k]
    attn = softmax_k(scores)
    out[b,h,w,q] = sum_k attn[b,q,k] * V[b,h,w,k]

Full inputs are [16, 128, 128, 64] f32. Data-parallel over batch across
8 NeuronCores (2 batches per core); no cross-core communication.

HBM-bound. Design notes (v5 schedule):
  - Q, K fp16 (score abs err ~0.09 vs ~45 typical top-2 gap), V fp8e3m4
    at scale 2 (total rel err ~1.37e-2 vs 2e-2 gate, deterministic),
    out bf16. Per-core traffic: 8 MiB QK + 2 MiB V loads + 4 MiB
    stores = 14 MiB.
  - Load rings: sync HWDGE + gpsimd SWDGE, both streaming from ~8.5us.
    The scalar HWDGE ring starts ~3us late (ACT-table DMA contention),
    so it carries NO loads and becomes the batch-0 store ring instead.
  - Ring order [qk0 pieces, v, qk1 pieces]: A0 chases 0.5MiB q0/k0
    pieces, V lands mid-stream so C0 can run right after sm0, and the
    last q1/k1 pieces are small so A1 ends right after the final load
    receipt (~2.5us after last byte).
  - PE stream: A0(chasing) [dummies] T0 C0 A1(chasing) [dummies] T1
    C1(+stores). Dummy matmuls bridge the sm0/sm1 ACT/DVE windows so
    HAM never sees a >3us PE idle gap (which would downgrade the PE to
    K=4/8 half duty for the next ~3-10us).
  - Softmax fused: tensor_tensor_reduce computes y=-(g_even+g_odd) and
    min-accumulates m=-max in one DVE op; exp(scale=-1, bias=m) on ACT
    with sum accumulator; attn = e/ssum via per-partition divide. The
    1/V_SCALE folds into the two bd scale-casts (DVE + ACT parallel).
  - Batch-1 output pieces stream out as produced on sync/gpsimd; the
    last piece is split across two rings to halve the store tail.
  - tc.tile_wait_until fences pin the Tile scheduler's static order;
    gaps must dominate its DMA-model error or it reorders the PE
    stream (hoisting A1 before C0 -> serialized tail + HAM throttle).
"""

import sys

sys.path.insert(0, "/opt/trn_rl_repo")

import ml_dtypes
import numpy as np

_B, _H, _W, _C = 16, 128, 128, 64
_NCORES = 8
_BPC = _B // _NCORES  # batches per core
_PAIRS = _W // 2  # w-pairs per batch

_V_SCALE = 2.0  # host-side V scale into fp8e3m4 range (max |2v| ~ 11.4 < 15.5)

_SPIECE = 32  # w-columns per batch-1 output store piece (0.5 MiB bf16)
_NSP = _W // _SPIECE

# Per-tensor load piece boundaries (w-pairs). Q pieces chase-grained;
# K pieces coarse (the sync ring runs ahead). The last q1 piece is
# smallest: its completion receipt gates A1's tail -> sm1 -> C1.
_Q_PIECES = {0: [(0, 32), (32, 48), (48, 64)], 1: [(0, 32), (32, 48), (48, 56), (56, 64)]}
_K_PIECES = {0: [(0, 32), (32, 64)], 1: [(0, 32), (32, 48), (48, 64)]}
_N_DUMMY = 100  # PE warm-keeper matmuls (~25ns each) bridging softmax gaps

_cache = {}


def _build_nc():
    from contextlib import ExitStack

    import concourse.bass as bass  # noqa: F401
    import concourse.tile as tile
    from concourse import bacc, mybir
    from concourse.masks import make_identity

    f32 = mybir.dt.float32
    f16 = mybir.dt.float16
    bf16 = mybir.dt.bfloat16
    f8 = mybir.dt.float8e3
    nc = bacc.Bacc(target_bir_lowering=False)

    q_ext = nc.declare_dram_parameter(
        "q16", [_BPC, _H, _PAIRS, 2 * _C], f16, isOutput=False
    )
    k_ext = nc.declare_dram_parameter(
        "k16", [_BPC, _H, _PAIRS, 2 * _C], f16, isOutput=False
    )
    vt_ext = nc.declare_dram_parameter(
        "v8t", [_BPC, 2 * _C, _PAIRS, _H], f8, isOutput=False
    )
    o_ext = nc.declare_dram_parameter("out", [_BPC, _H, _W, _C], bf16, isOutput=True)

    with tile.TileContext(nc) as tc, ExitStack() as ctx:
        singles = ctx.enter_context(tc.tile_pool(name="singles", bufs=1))
        qp = ctx.enter_context(tc.tile_pool(name="qp", bufs=1))
        kp = ctx.enter_context(tc.tile_pool(name="kp", bufs=1))
        vp = ctx.enter_context(tc.tile_pool(name="vp", bufs=2))
        op0p = ctx.enter_context(tc.tile_pool(name="op0p", bufs=1))
        op = ctx.enter_context(tc.tile_pool(name="op", bufs=4))
        sm = ctx.enter_context(tc.tile_pool(name="sm", bufs=2))
        ps_sc = ctx.enter_context(tc.tile_pool(name="ps_sc", bufs=1, space="PSUM"))
        ps_at = ctx.enter_context(tc.tile_pool(name="ps_at", bufs=1, space="PSUM"))
        ps_o = ctx.enter_context(tc.tile_pool(name="ps_o", bufs=5, space="PSUM"))
        ps_w = ctx.enter_context(tc.tile_pool(name="ps_w", bufs=1, space="PSUM"))

        # ---- tiles
        qt, kt = {0: [], 1: []}, {0: [], 1: []}
        for b in (0, 1):
            for i, (lo, hi) in enumerate(_Q_PIECES[b]):
                t = qp.tile([_H, hi - lo, 2 * _C], f16, tag=f"qt{b}{i}", name=f"qt{b}{i}")
                qt[b].append((t, lo, hi))
            for i, (lo, hi) in enumerate(_K_PIECES[b]):
                t = kp.tile([_H, hi - lo, 2 * _C], f16, tag=f"kt{b}{i}", name=f"kt{b}{i}")
                kt[b].append((t, lo, hi))
        vt = {
            b: vp.tile([2 * _C, _PAIRS, _H], f8, tag="vt", name=f"vt{b}")
            for b in (0, 1)
        }
        out0 = op0p.tile([_H, _W, _C], bf16, tag="out0", name="out0")
        bd = {
            b: sm.tile([2 * _C, 2, _C], f16, tag=f"bd{b}", name=f"bd{b}")
            for b in (0, 1)
        }
        warm = singles.tile([2 * _C, 8], f16)

        # ---- loads: ring FIFO order is the schedule. Both HWDGE rings
        # (SWDGE loads measured ~40% slower). Scalar starts ~2-3us late
        # (ACT-table DMA) so it carries less. V rides mid-stream (v0
        # right after q0 so C0 can start at bd0-ready); the last q1
        # piece is smallest and gates only A1's 8-pair tail.
        with tc.tile_wait_until(0.000):
            # scalar (slow ring, ~170 GB/s + late start, 4.25 MiB):
            #   q0a q0b q1a q1b q1c v1
            # sync (fast ring, 5.75 MiB):
            #   k0a k0b q0c v0 k1a k1b k1c q1d
            # Issues interleaved scalar/sync so the 8 DMA-completion
            # semaphore lanes only ever pair same-ring FIFO-ordered
            # transfers (a cross-ring pair can make a consumer wait on
            # an unrelated later DMA).
            # Ring FIFOs — scalar: q0a q0b q1a q1b q1c v1 (4.25 MiB);
            # sync: v0 k0a k0b q0c k1a k1b k1c q1d (5.75 MiB).
            # Each issue gets its own ascending micro-fence so the Tile
            # scheduler's virtual issue order (which drives the 8-lane
            # DMA-semaphore round-robin) exactly matches ring order:
            # lane sharing then always pairs FIFO-ordered transfers and
            # a consumer's wait can never couple to a LATER unrelated
            # DMA (that cost C0 ~5us when v0 shared a lane with k1).
            def qsrc(b, i):
                t, lo, hi = qt[b][i]
                return t, q_ext[b, :, lo:hi, :]

            def ksrc(b, i):
                t, lo, hi = kt[b][i]
                return t, k_ext[b, :, lo:hi, :]

            issues = [
                (nc.sync, vt[0], vt_ext[0]),
                (nc.scalar, *qsrc(0, 0)),
                (nc.sync, *ksrc(0, 0)),
                (nc.scalar, *qsrc(0, 1)),
                (nc.sync, *ksrc(0, 1)),
                (nc.scalar, *qsrc(1, 0)),
                (nc.sync, *qsrc(0, 2)),
                (nc.scalar, *qsrc(1, 1)),
                (nc.sync, *ksrc(1, 0)),
                (nc.scalar, *qsrc(1, 2)),
                (nc.sync, *ksrc(1, 1)),
                (nc.scalar, vt[1], vt_ext[1]),
                (nc.sync, *ksrc(1, 2)),
                (nc.sync, *qsrc(1, 3)),
            ]
            for n, (eng, tile_out, src) in enumerate(issues):
                with tc.tile_wait_until(0.0001 * (n + 1)):
                    eng.dma_start(out=tile_out, in_=src)

            ident = singles.tile([_C, _C], f32)
            make_identity(nc, ident)
            nc.vector.memset(warm, 0.0)
            nc.vector.memset(bd[0], 0.0)
            nc.vector.memset(bd[1], 0.0)

        def emit_phase_a(b):
            gram = ps_sc.tile([2 * _C, 2, _C], f32, tag="gram")
            for j in range(_PAIRS):
                q_t, qlo, _ = next(x for x in qt[b] if x[1] <= j < x[2])
                k_t, klo, _ = next(x for x in kt[b] if x[1] <= j < x[2])
                nc.tensor.matmul(
                    gram,
                    lhsT=q_t[:, j - qlo, :],
                    rhs=k_t[:, j - klo, :],
                    start=(j == 0),
                    stop=(j == _PAIRS - 1),
                )
            return gram

        def emit_dummies():
            # PE warm-keepers bridging a softmax ACT/DVE window so HAM
            # never sees a long PE idle gap (=> K=4/8 for the next phase)
            wps = ps_w.tile([8, 8], f32, tag="wps")
            for _ in range(_N_DUMMY):
                nc.tensor.matmul(wps, lhsT=warm[:, 0:8], rhs=warm, start=True, stop=True)

        def emit_softmax(gram):
            # scores = even-w block + odd-w block of the pair Gram tile
            # (one operand staged to SBUF: one PSUM read per instruction)
            s0 = sm.tile([_C, _C], f32, tag="s0")
            nc.vector.tensor_copy(out=s0, in_=gram[0:_C, 0, :])
            scores = sm.tile([_C, _C], f32, tag="scores")
            nc.vector.tensor_tensor(
                out=scores,
                in0=gram[_C : 2 * _C, 1, :],
                in1=s0,
                op=mybir.AluOpType.add,
            )
            negmax = sm.tile([_C, 1], f32, tag="negmax")
            nc.vector.tensor_reduce(
                out=negmax,
                in_=scores,
                axis=mybir.AxisListType.X,
                op=mybir.AluOpType.max,
                negate=True,
            )
            e = sm.tile([_C, _C], f32, tag="e")
            ssum = sm.tile([_C, 1], f32, tag="ssum")
            nc.scalar.activation(
                out=e,
                in_=scores,
                func=mybir.ActivationFunctionType.Exp,
                bias=negmax,
                scale=1.0,
                accum_out=ssum,
            )
            rsum = sm.tile([_C, 1], f32, tag="rsum")
            nc.vector.reciprocal(out=rsum, in_=ssum)
            attn = sm.tile([_C, _C], f32, tag="attn")
            nc.vector.tensor_scalar_mul(attn, e, rsum)
            return attn

        def emit_bd(attn, b):
            # PE transpose, then block-diag bd with the 1/V_SCALE folded
            # into the two scale-casts (DVE + ACT in parallel)
            attnT_ps = ps_at.tile([_C, _C], f32, tag="attnT_ps")
            nc.tensor.transpose(attnT_ps, attn, ident)
            nc.vector.tensor_scalar_mul(
                bd[b][0:_C, 0, :], attnT_ps, float(1.0 / _V_SCALE)
            )
            nc.scalar.activation(
                out=bd[b][_C : 2 * _C, 1, :],
                in_=attnT_ps,
                func=mybir.ActivationFunctionType.Copy,
                scale=float(1.0 / _V_SCALE),
            )

        def emit_c_chunks(b, otile, wbase, p_lo, p_hi, par):
            """matmul+copy chunks for pairs [p_lo, p_hi) of batch b into
            otile starting at w offset (p_lo*2 - wbase)."""
            for wg in range(p_lo, p_hi, 4):  # 4 pairs per PSUM bank
                o_ps = ps_o.tile([_H, 8, _C], f32, tag="o_ps")
                for half in range(4):
                    nc.tensor.matmul(
                        o_ps[:, 2 * half : 2 * half + 2, :],
                        lhsT=vt[b][:, wg + half, :],
                        rhs=bd[b],
                        start=True,
                        stop=True,
                    )
                w0 = wg * 2 - wbase
                dst = otile[:, w0 : w0 + 8, :]
                if (wg // 4 + par) % 2 == 0:
                    nc.vector.tensor_copy(out=dst, in_=o_ps)
                else:
                    nc.scalar.activation(
                        out=dst, in_=o_ps, func=mybir.ActivationFunctionType.Copy
                    )

        # ---- the pinned chain
        with tc.tile_wait_until(0.010):
            gram0 = emit_phase_a(0)
        with tc.tile_wait_until(0.030):
            attn0 = emit_softmax(gram0)
            emit_dummies()
            emit_bd(attn0, 0)
        with tc.tile_wait_until(0.040):
            # C0: whole batch-0 output into SBUF; halves stored behind
            # the loads in each HWDGE ring's FIFO (SWDGE stores crawl
            # when issued while the HW rings still stream loads).
            emit_c_chunks(0, out0, 0, 0, 32, 0)
            nc.sync.dma_start(out=o_ext[0, :, 0:64, :], in_=out0[:, 0:64, :])
        with tc.tile_wait_until(0.045):
            emit_c_chunks(0, out0, 0, 32, 64, 0)
            nc.scalar.dma_start(out=o_ext[0, :, 64:128, :], in_=out0[:, 64:128, :])
        with tc.tile_wait_until(0.060):
            gram1 = emit_phase_a(1)
        with tc.tile_wait_until(0.070):
            attn1 = emit_softmax(gram1)
            emit_dummies()
            emit_bd(attn1, 1)
        with tc.tile_wait_until(0.090):
            # C1: produce + store pieces as they complete
            store_rings = [
                [nc.sync],
                [nc.scalar],
                [nc.sync],
                [nc.sync, nc.scalar],
            ]
            for pc in range(_NSP):
                otile = op.tile([_H, _SPIECE, _C], bf16, tag="otile")
                emit_c_chunks(
                    1, otile, pc * _SPIECE, pc * (_SPIECE // 2), (pc + 1) * (_SPIECE // 2), pc
                )
                w0 = pc * _SPIECE
                rings = store_rings[pc]
                wstep = _SPIECE // len(rings)
                for ri, eng in enumerate(rings):
                    sl = slice(w0 + ri * wstep, w0 + (ri + 1) * wstep)
                    eng.dma_start(
                        out=o_ext[1, :, sl, :],
                        in_=otile[:, ri * wstep : (ri + 1) * wstep, :],
                    )

    nc.finalize()
    return nc


def _get_nc():
    if "nc" not in _cache:
        _cache["nc"] = _build_nc()
    return _cache["nc"]


def _prep_inputs(q, k, v):
    """Host-side layout prep: fp16 casts of Q/K, V scaled into fp8e3m4
    and transposed per w-pair."""
    q16 = q.astype(np.float16).reshape(_B, _H, _PAIRS, 2 * _C)
    k16 = k.astype(np.float16).reshape(_B, _H, _PAIRS, 2 * _C)
    v8 = (v * _V_SCALE).astype(ml_dtypes.float8_e3m4)  # [B, H, W, C]
    # vt[b, (dw c), j, h] = v[b, h, 2j+dw, c]
    x = v8.transpose(0, 2, 3, 1)  # [B, W, C, H]
    x = x.reshape(_B, _PAIRS, 2, _C, _H)  # [B, j, dw, C, H]
    vt = np.ascontiguousarray(x.transpose(0, 2, 3, 1, 4)).reshape(
        _B, 2 * _C, _PAIRS, _H
    )
    return q16, k16, vt


def run(inputs, trace=False):
    """Run the SPMD kernel. Returns (full_output, BassKernelResults)."""
    from concourse.bass_utils import run_bass_kernel_spmd

    q = np.asarray(inputs["query"], dtype=np.float32)
    k = np.asarray(inputs["keys"], dtype=np.float32)
    v = np.asarray(inputs["values"], dtype=np.float32)
    assert q.shape == (_B, _H, _W, _C), q.shape

    q16, k16, vt = _prep_inputs(q, k, v)

    nc = _get_nc()
    in_maps = []
    for i in range(_NCORES):
        sl = slice(i * _BPC, (i + 1) * _BPC)
        in_maps.append({"q16": q16[sl], "k16": k16[sl], "v8t": vt[sl]})

    res = run_bass_kernel_spmd(
        nc, in_maps, core_ids=list(range(_NCORES)), trace=trace
    )
    out = np.concatenate(
        [res.results[i]["out"].astype(np.float32) for i in range(_NCORES)], axis=0
    )
    return out, res


def kernel(**inputs) -> np.ndarray:
    out, _ = run(inputs, trace=False)
    return out


# revision 42
# speedup vs baseline: 1.0712x; 1.0712x over previous
"""Trainium2 Bass kernel for channel-attention:
    scores[b,q,k] = sum_{h,w} Q[b,h,w,q] * K[b,h,w,k]
    attn = softmax_k(scores)
    out[b,h,w,q] = sum_k attn[b,q,k] * V[b,h,w,k]

Full inputs are [16, 128, 128, 64] f32. Data-parallel over batch across
8 NeuronCores (2 batches per core); no cross-core communication.

HBM-bound. Design notes (v5 schedule):
  - Q, K fp16 (score abs err ~0.09 vs ~45 typical top-2 gap), V fp8e3m4
    at scale 2 (total rel err ~1.37e-2 vs 2e-2 gate, deterministic),
    out bf16. Per-core traffic: 8 MiB QK + 2 MiB V loads + 4 MiB
    stores = 14 MiB.
  - Load rings: sync HWDGE + gpsimd SWDGE, both streaming from ~8.5us.
    The scalar HWDGE ring starts ~3us late (ACT-table DMA contention),
    so it carries NO loads and becomes the batch-0 store ring instead.
  - Ring order [qk0 pieces, v, qk1 pieces]: A0 chases 0.5MiB q0/k0
    pieces, V lands mid-stream so C0 can run right after sm0, and the
    last q1/k1 pieces are small so A1 ends right after the final load
    receipt (~2.5us after last byte).
  - PE stream: A0(chasing) [dummies] T0 C0 A1(chasing) [dummies] T1
    C1(+stores). Dummy matmuls bridge the sm0/sm1 ACT/DVE windows so
    HAM never sees a >3us PE idle gap (which would downgrade the PE to
    K=4/8 half duty for the next ~3-10us).
  - Softmax fused: tensor_tensor_reduce computes y=-(g_even+g_odd) and
    min-accumulates m=-max in one DVE op; exp(scale=-1, bias=m) on ACT
    with sum accumulator; attn = e/ssum via per-partition divide. The
    1/V_SCALE folds into the two bd scale-casts (DVE + ACT parallel).
  - Batch-1 output pieces stream out as produced on sync/gpsimd; the
    last piece is split across two rings to halve the store tail.
  - tc.tile_wait_until fences pin the Tile scheduler's static order;
    gaps must dominate its DMA-model error or it reorders the PE
    stream (hoisting A1 before C0 -> serialized tail + HAM throttle).
"""

import sys

sys.path.insert(0, "/opt/trn_rl_repo")

import ml_dtypes
import numpy as np

_B, _H, _W, _C = 16, 128, 128, 64
_NCORES = 8
_BPC = _B // _NCORES  # batches per core
_PAIRS = _W // 2  # w-pairs per batch

_V_SCALE = 2.0  # host-side V scale into fp8e3m4 range (max |2v| ~ 11.4 < 15.5)

_SPIECE = 32  # w-columns per batch-1 output store piece (0.5 MiB bf16)
_NSP = _W // _SPIECE

# Per-tensor load piece boundaries (w-pairs). Q pieces chase-grained;
# K pieces coarse (the sync ring runs ahead). The last q1 piece is
# smallest: its completion receipt gates A1's tail -> sm1 -> C1.
_Q_PIECES = {0: [(0, 24), (24, 48), (48, 64)], 1: [(0, 32), (32, 48), (48, 56), (56, 64)]}
_K_PIECES = {0: [(0, 32), (32, 64)], 1: [(0, 32), (32, 64)]}
_N_DUMMY = 36  # PE warm-keeper matmuls (~25ns each) bridging softmax gaps

_cache = {}


def _build_nc():
    from contextlib import ExitStack

    import concourse.bass as bass  # noqa: F401
    import concourse.tile as tile
    from concourse import bacc, mybir
    from concourse.masks import make_identity

    f32 = mybir.dt.float32
    f16 = mybir.dt.float16
    bf16 = mybir.dt.bfloat16
    f8 = mybir.dt.float8e3
    nc = bacc.Bacc(target_bir_lowering=False)

    q_ext = nc.declare_dram_parameter(
        "q16", [_BPC, _H, _PAIRS, 2 * _C], f16, isOutput=False
    )
    k_ext = nc.declare_dram_parameter(
        "k16", [_BPC, _H, _PAIRS, 2 * _C], f16, isOutput=False
    )
    vt_ext = nc.declare_dram_parameter(
        "v8t", [_BPC, 2 * _C, _PAIRS, _H], f8, isOutput=False
    )
    o_ext = nc.declare_dram_parameter("out", [_BPC, _H, _W, _C], bf16, isOutput=True)

    with tile.TileContext(nc) as tc, ExitStack() as ctx:
        singles = ctx.enter_context(tc.tile_pool(name="singles", bufs=1))
        qp = ctx.enter_context(tc.tile_pool(name="qp", bufs=1))
        kp = ctx.enter_context(tc.tile_pool(name="kp", bufs=1))
        vp = ctx.enter_context(tc.tile_pool(name="vp", bufs=2))
        op0p = ctx.enter_context(tc.tile_pool(name="op0p", bufs=1))
        op = ctx.enter_context(tc.tile_pool(name="op", bufs=4))
        sm = ctx.enter_context(tc.tile_pool(name="sm", bufs=2))
        ps_sc = ctx.enter_context(tc.tile_pool(name="ps_sc", bufs=2, space="PSUM"))
        ps_at = ctx.enter_context(tc.tile_pool(name="ps_at", bufs=1, space="PSUM"))
        ps_o = ctx.enter_context(tc.tile_pool(name="ps_o", bufs=4, space="PSUM"))
        ps_w = ctx.enter_context(tc.tile_pool(name="ps_w", bufs=1, space="PSUM"))

        # ---- tiles
        qt, kt = {0: [], 1: []}, {0: [], 1: []}
        for b in (0, 1):
            for i, (lo, hi) in enumerate(_Q_PIECES[b]):
                t = qp.tile([_H, hi - lo, 2 * _C], f16, tag=f"qt{b}{i}", name=f"qt{b}{i}")
                qt[b].append((t, lo, hi))
            for i, (lo, hi) in enumerate(_K_PIECES[b]):
                t = kp.tile([_H, hi - lo, 2 * _C], f16, tag=f"kt{b}{i}", name=f"kt{b}{i}")
                kt[b].append((t, lo, hi))
        vt = {
            b: vp.tile([2 * _C, _PAIRS, _H], f8, tag="vt", name=f"vt{b}")
            for b in (0, 1)
        }
        out0 = op0p.tile([_H, _W, _C], bf16, tag="out0", name="out0")
        bd = {
            b: sm.tile([2 * _C, 2, _C], f16, tag=f"bd{b}", name=f"bd{b}")
            for b in (0, 1)
        }
        warm = singles.tile([2 * _C, 8], f16)

        # ---- loads: ring FIFO order is the schedule. Both HWDGE rings
        # (SWDGE loads measured ~40% slower). Scalar starts ~2-3us late
        # (ACT-table DMA) so it carries less. V rides mid-stream (v0
        # right after q0 so C0 can start at bd0-ready); the last q1
        # piece is smallest and gates only A1's 8-pair tail.
        with tc.tile_wait_until(0.000):
            # scalar (slow ring, ~170 GB/s + late start, 4.25 MiB):
            #   q0a q0b q1a q1b q1c v1
            # sync (fast ring, 5.75 MiB):
            #   k0a k0b q0c v0 k1a k1b k1c q1d
            # Issues interleaved scalar/sync so the 8 DMA-completion
            # semaphore lanes only ever pair same-ring FIFO-ordered
            # transfers (a cross-ring pair can make a consumer wait on
            # an unrelated later DMA).
            # Ring FIFOs — scalar: q0a q0b q1a q1b q1c v1 (4.25 MiB);
            # sync: v0 k0a k0b q0c k1a k1b k1c q1d (5.75 MiB).
            # Each issue gets its own ascending micro-fence so the Tile
            # scheduler's virtual issue order (which drives the 8-lane
            # DMA-semaphore round-robin) exactly matches ring order:
            # lane sharing then always pairs FIFO-ordered transfers and
            # a consumer's wait can never couple to a LATER unrelated
            # DMA (that cost C0 ~5us when v0 shared a lane with k1).
            # scalar: q0a q0b q0c q1a q1b q1c v1  (4.75 MiB)
            for i in range(3):
                t, lo, hi = qt[0][i]
                nc.scalar.dma_start(out=t, in_=q_ext[0, :, lo:hi, :])
            for i in range(3):
                t, lo, hi = qt[1][i]
                nc.scalar.dma_start(out=t, in_=q_ext[1, :, lo:hi, :])
            nc.scalar.dma_start(out=vt[1], in_=vt_ext[1])
            # sync: k0a k0b v0 k1a k1b q1d  (5.25 MiB) — v0 early so C0
            # can start right at bd0-ready
            for t, lo, hi in kt[0]:
                nc.sync.dma_start(out=t, in_=k_ext[0, :, lo:hi, :])
            nc.sync.dma_start(out=vt[0], in_=vt_ext[0])
            for t, lo, hi in kt[1]:
                nc.sync.dma_start(out=t, in_=k_ext[1, :, lo:hi, :])
            t, lo, hi = qt[1][3]
            nc.sync.dma_start(out=t, in_=q_ext[1, :, lo:hi, :])

            ident = singles.tile([_C, _C], f32)
            make_identity(nc, ident)
            nc.vector.memset(warm, 0.0)
            nc.vector.memset(bd[0], 0.0)
            nc.vector.memset(bd[1], 0.0)

        def emit_phase_a(b):
            gram = ps_sc.tile([2 * _C, 2, _C], f32, tag="gram")
            for j in range(_PAIRS):
                q_t, qlo, _ = next(x for x in qt[b] if x[1] <= j < x[2])
                k_t, klo, _ = next(x for x in kt[b] if x[1] <= j < x[2])
                nc.tensor.matmul(
                    gram,
                    lhsT=q_t[:, j - qlo, :],
                    rhs=k_t[:, j - klo, :],
                    start=(j == 0),
                    stop=(j == _PAIRS - 1),
                )
            return gram

        def emit_dummies():
            # PE warm-keepers bridging a softmax ACT/DVE window so HAM
            # never sees a long PE idle gap (=> K=4/8 for the next phase)
            wps = ps_w.tile([8, 8], f32, tag="wps")
            for _ in range(_N_DUMMY):
                nc.tensor.matmul(wps, lhsT=warm[:, 0:8], rhs=warm, start=True, stop=True)

        def emit_softmax(gram):
            # scores = even-w block + odd-w block of the pair Gram tile
            # (one operand staged to SBUF: one PSUM read per instruction)
            s0 = sm.tile([_C, _C], f32, tag="s0")
            nc.vector.tensor_copy(out=s0, in_=gram[0:_C, 0, :])
            scores = sm.tile([_C, _C], f32, tag="scores")
            nc.vector.tensor_tensor(
                out=scores,
                in0=gram[_C : 2 * _C, 1, :],
                in1=s0,
                op=mybir.AluOpType.add,
            )
            negmax = sm.tile([_C, 1], f32, tag="negmax")
            nc.vector.tensor_reduce(
                out=negmax,
                in_=scores,
                axis=mybir.AxisListType.X,
                op=mybir.AluOpType.max,
                negate=True,
            )
            e = sm.tile([_C, _C], f32, tag="e")
            ssum = sm.tile([_C, 1], f32, tag="ssum")
            nc.scalar.activation(
                out=e,
                in_=scores,
                func=mybir.ActivationFunctionType.Exp,
                bias=negmax,
                scale=1.0,
                accum_out=ssum,
            )
            rsum = sm.tile([_C, 1], f32, tag="rsum")
            nc.vector.reciprocal(out=rsum, in_=ssum)
            attn = sm.tile([_C, _C], f32, tag="attn")
            nc.vector.tensor_scalar_mul(attn, e, rsum)
            return attn

        def emit_bd(attn, b):
            # PE transpose, then block-diag bd with the 1/V_SCALE folded
            # into the two scale-casts (DVE + ACT in parallel)
            attnT_ps = ps_at.tile([_C, _C], f32, tag="attnT_ps")
            nc.tensor.transpose(attnT_ps, attn, ident)
            nc.vector.tensor_scalar_mul(
                bd[b][0:_C, 0, :], attnT_ps, float(1.0 / _V_SCALE)
            )
            nc.scalar.activation(
                out=bd[b][_C : 2 * _C, 1, :],
                in_=attnT_ps,
                func=mybir.ActivationFunctionType.Copy,
                scale=float(1.0 / _V_SCALE),
            )

        def emit_c_chunks(b, otile, wbase, p_lo, p_hi, par):
            """matmul+copy chunks for pairs [p_lo, p_hi) of batch b into
            otile starting at w offset (p_lo*2 - wbase)."""
            for wg in range(p_lo, p_hi, 4):  # 4 pairs per PSUM bank
                o_ps = ps_o.tile([_H, 8, _C], f32, tag="o_ps")
                for half in range(4):
                    nc.tensor.matmul(
                        o_ps[:, 2 * half : 2 * half + 2, :],
                        lhsT=vt[b][:, wg + half, :],
                        rhs=bd[b],
                        start=True,
                        stop=True,
                    )
                w0 = wg * 2 - wbase
                dst = otile[:, w0 : w0 + 8, :]
                if (wg // 4 + par) % 2 == 0:
                    nc.vector.tensor_copy(out=dst, in_=o_ps)
                else:
                    nc.scalar.activation(
                        out=dst, in_=o_ps, func=mybir.ActivationFunctionType.Copy
                    )

        # ---- the pinned chain
        with tc.tile_wait_until(0.010):
            gram0 = emit_phase_a(0)
        with tc.tile_wait_until(0.030):
            attn0 = emit_softmax(gram0)
            emit_dummies()
            emit_bd(attn0, 0)
        with tc.tile_wait_until(0.040):
            # C0: whole batch-0 output into SBUF; halves stored behind
            # the loads in each HWDGE ring's FIFO (SWDGE stores crawl
            # when issued while the HW rings still stream loads).
            emit_c_chunks(0, out0, 0, 0, 32, 0)
            nc.gpsimd.dma_start(out=o_ext[0, :, 0:64, :], in_=out0[:, 0:64, :])
        with tc.tile_wait_until(0.045):
            emit_c_chunks(0, out0, 0, 32, 64, 0)
            nc.scalar.dma_start(out=o_ext[0, :, 64:128, :], in_=out0[:, 64:128, :])
        with tc.tile_wait_until(0.060):
            gram1 = emit_phase_a(1)
        with tc.tile_wait_until(0.070):
            attn1 = emit_softmax(gram1)
            emit_dummies()
            emit_bd(attn1, 1)
        with tc.tile_wait_until(0.090):
            # C1: produce + store pieces as they complete
            store_rings = [
                [nc.sync],
                [nc.scalar],
                [nc.gpsimd],
                [nc.sync, nc.scalar],
            ]
            for pc in range(_NSP):
                otile = op.tile([_H, _SPIECE, _C], bf16, tag="otile")
                emit_c_chunks(
                    1, otile, pc * _SPIECE, pc * (_SPIECE // 2), (pc + 1) * (_SPIECE // 2), pc
                )
                w0 = pc * _SPIECE
                rings = store_rings[pc]
                wstep = _SPIECE // len(rings)
                for ri, eng in enumerate(rings):
                    sl = slice(w0 + ri * wstep, w0 + (ri + 1) * wstep)
                    eng.dma_start(
                        out=o_ext[1, :, sl, :],
                        in_=otile[:, ri * wstep : (ri + 1) * wstep, :],
                    )

    nc.finalize()
    return nc


def _get_nc():
    if "nc" not in _cache:
        _cache["nc"] = _build_nc()
    return _cache["nc"]


def _prep_inputs(q, k, v):
    """Host-side layout prep: fp16 casts of Q/K, V scaled into fp8e3m4
    and transposed per w-pair."""
    q16 = q.astype(np.float16).reshape(_B, _H, _PAIRS, 2 * _C)
    k16 = k.astype(np.float16).reshape(_B, _H, _PAIRS, 2 * _C)
    v8 = (v * _V_SCALE).astype(ml_dtypes.float8_e3m4)  # [B, H, W, C]
    # vt[b, (dw c), j, h] = v[b, h, 2j+dw, c]
    x = v8.transpose(0, 2, 3, 1)  # [B, W, C, H]
    x = x.reshape(_B, _PAIRS, 2, _C, _H)  # [B, j, dw, C, H]
    vt = np.ascontiguousarray(x.transpose(0, 2, 3, 1, 4)).reshape(
        _B, 2 * _C, _PAIRS, _H
    )
    return q16, k16, vt


def run(inputs, trace=False):
    """Run the SPMD kernel. Returns (full_output, BassKernelResults)."""
    from concourse.bass_utils import run_bass_kernel_spmd

    q = np.asarray(inputs["query"], dtype=np.float32)
    k = np.asarray(inputs["keys"], dtype=np.float32)
    v = np.asarray(inputs["values"], dtype=np.float32)
    assert q.shape == (_B, _H, _W, _C), q.shape

    q16, k16, vt = _prep_inputs(q, k, v)

    nc = _get_nc()
    in_maps = []
    for i in range(_NCORES):
        sl = slice(i * _BPC, (i + 1) * _BPC)
        in_maps.append({"q16": q16[sl], "k16": k16[sl], "v8t": vt[sl]})

    res = run_bass_kernel_spmd(
        nc, in_maps, core_ids=list(range(_NCORES)), trace=trace
    )
    out = np.concatenate(
        [res.results[i]["out"].astype(np.float32) for i in range(_NCORES)], axis=0
    )
    return out, res


def kernel(**inputs) -> np.ndarray:
    out, _ = run(inputs, trace=False)
    return out


# revision 43
# speedup vs baseline: 1.1145x; 1.0404x over previous
"""Trainium2 Bass kernel for channel-attention:
    scores[b,q,k] = sum_{h,w} Q[b,h,w,q] * K[b,h,w,k]
    attn = softmax_k(scores)
    out[b,h,w,q] = sum_k attn[b,q,k] * V[b,h,w,k]

Full inputs are [16, 128, 128, 64] f32. Data-parallel over batch across
8 NeuronCores (2 batches per core); no cross-core communication.

HBM-bound. Design notes (v5 schedule):
  - Q, K fp16 (score abs err ~0.09 vs ~45 typical top-2 gap), V fp8e3m4
    at scale 2 (total rel err ~1.37e-2 vs 2e-2 gate, deterministic),
    out bf16. Per-core traffic: 8 MiB QK + 2 MiB V loads + 4 MiB
    stores = 14 MiB.
  - Load rings: sync HWDGE + gpsimd SWDGE, both streaming from ~8.5us.
    The scalar HWDGE ring starts ~3us late (ACT-table DMA contention),
    so it carries NO loads and becomes the batch-0 store ring instead.
  - Ring order [qk0 pieces, v, qk1 pieces]: A0 chases 0.5MiB q0/k0
    pieces, V lands mid-stream so C0 can run right after sm0, and the
    last q1/k1 pieces are small so A1 ends right after the final load
    receipt (~2.5us after last byte).
  - PE stream: A0(chasing) [dummies] T0 C0 A1(chasing) [dummies] T1
    C1(+stores). Dummy matmuls bridge the sm0/sm1 ACT/DVE windows so
    HAM never sees a >3us PE idle gap (which would downgrade the PE to
    K=4/8 half duty for the next ~3-10us).
  - Softmax fused: tensor_tensor_reduce computes y=-(g_even+g_odd) and
    min-accumulates m=-max in one DVE op; exp(scale=-1, bias=m) on ACT
    with sum accumulator; attn = e/ssum via per-partition divide. The
    1/V_SCALE folds into the two bd scale-casts (DVE + ACT parallel).
  - Batch-1 output pieces stream out as produced on sync/gpsimd; the
    last piece is split across two rings to halve the store tail.
  - tc.tile_wait_until fences pin the Tile scheduler's static order;
    gaps must dominate its DMA-model error or it reorders the PE
    stream (hoisting A1 before C0 -> serialized tail + HAM throttle).
"""

import sys

sys.path.insert(0, "/opt/trn_rl_repo")

import ml_dtypes
import numpy as np

_B, _H, _W, _C = 16, 128, 128, 64
_NCORES = 8
_BPC = _B // _NCORES  # batches per core
_PAIRS = _W // 2  # w-pairs per batch

_V_SCALE = 2.0  # host-side V scale into fp8e3m4 range (max |2v| ~ 11.4 < 15.5)

_SPIECE = 32  # w-columns per batch-1 output store piece (0.5 MiB bf16)
_NSP = _W // _SPIECE

# Per-tensor load piece boundaries (w-pairs). Q pieces chase-grained;
# K pieces coarse (the sync ring runs ahead). The last q1 piece is
# smallest: its completion receipt gates A1's tail -> sm1 -> C1.
_Q_PIECES = {0: [(0, 24), (24, 48), (48, 64)], 1: [(0, 32), (32, 48), (48, 56), (56, 64)]}
_K_PIECES = {0: [(0, 32), (32, 64)], 1: [(0, 32), (32, 64)]}
_N_DUMMY = 36  # PE warm-keeper matmuls (~25ns each) bridging softmax gaps

_cache = {}


def _build_nc():
    from contextlib import ExitStack

    import concourse.bass as bass  # noqa: F401
    import concourse.tile as tile
    from concourse import bacc, mybir
    from concourse.masks import make_identity

    f32 = mybir.dt.float32
    f16 = mybir.dt.float16
    bf16 = mybir.dt.bfloat16
    f8 = mybir.dt.float8e3
    nc = bacc.Bacc(target_bir_lowering=False)

    q_ext = nc.declare_dram_parameter(
        "q16", [_BPC, _H, _PAIRS, 2 * _C], f16, isOutput=False
    )
    k_ext = nc.declare_dram_parameter(
        "k16", [_BPC, _H, _PAIRS, 2 * _C], f16, isOutput=False
    )
    vt_ext = nc.declare_dram_parameter(
        "v8t", [_BPC, 2 * _C, _PAIRS, _H], f8, isOutput=False
    )
    o_ext = nc.declare_dram_parameter("out", [_BPC, _H, _W, _C], bf16, isOutput=True)

    with tile.TileContext(nc) as tc, ExitStack() as ctx:
        singles = ctx.enter_context(tc.tile_pool(name="singles", bufs=1))
        qp = ctx.enter_context(tc.tile_pool(name="qp", bufs=1))
        kp = ctx.enter_context(tc.tile_pool(name="kp", bufs=1))
        vp = ctx.enter_context(tc.tile_pool(name="vp", bufs=2))
        op0p = ctx.enter_context(tc.tile_pool(name="op0p", bufs=1))
        op = ctx.enter_context(tc.tile_pool(name="op", bufs=4))
        sm = ctx.enter_context(tc.tile_pool(name="sm", bufs=2))
        ps_sc = ctx.enter_context(tc.tile_pool(name="ps_sc", bufs=2, space="PSUM"))
        ps_at = ctx.enter_context(tc.tile_pool(name="ps_at", bufs=1, space="PSUM"))
        ps_o = ctx.enter_context(tc.tile_pool(name="ps_o", bufs=4, space="PSUM"))
        ps_w = ctx.enter_context(tc.tile_pool(name="ps_w", bufs=1, space="PSUM"))

        # ---- tiles
        qt, kt = {0: [], 1: []}, {0: [], 1: []}
        for b in (0, 1):
            for i, (lo, hi) in enumerate(_Q_PIECES[b]):
                t = qp.tile([_H, hi - lo, 2 * _C], f16, tag=f"qt{b}{i}", name=f"qt{b}{i}")
                qt[b].append((t, lo, hi))
            for i, (lo, hi) in enumerate(_K_PIECES[b]):
                t = kp.tile([_H, hi - lo, 2 * _C], f16, tag=f"kt{b}{i}", name=f"kt{b}{i}")
                kt[b].append((t, lo, hi))
        vt = {
            b: vp.tile([2 * _C, _PAIRS, _H], f8, tag="vt", name=f"vt{b}")
            for b in (0, 1)
        }
        out0 = op0p.tile([_H, _W, _C], bf16, tag="out0", name="out0")
        bd = {
            b: sm.tile([2 * _C, 2, _C], f16, tag=f"bd{b}", name=f"bd{b}")
            for b in (0, 1)
        }
        warm = singles.tile([2 * _C, 8], f16)

        # ---- loads: ring FIFO order is the schedule. Both HWDGE rings
        # (SWDGE loads measured ~40% slower). Scalar starts ~2-3us late
        # (ACT-table DMA) so it carries less. V rides mid-stream (v0
        # right after q0 so C0 can start at bd0-ready); the last q1
        # piece is smallest and gates only A1's 8-pair tail.
        with tc.tile_wait_until(0.000):
            # scalar (slow ring, ~170 GB/s + late start, 4.25 MiB):
            #   q0a q0b q1a q1b q1c v1
            # sync (fast ring, 5.75 MiB):
            #   k0a k0b q0c v0 k1a k1b k1c q1d
            # Issues interleaved scalar/sync so the 8 DMA-completion
            # semaphore lanes only ever pair same-ring FIFO-ordered
            # transfers (a cross-ring pair can make a consumer wait on
            # an unrelated later DMA).
            # Ring FIFOs — scalar: q0a q0b q1a q1b q1c v1 (4.25 MiB);
            # sync: v0 k0a k0b q0c k1a k1b k1c q1d (5.75 MiB).
            # Each issue gets its own ascending micro-fence so the Tile
            # scheduler's virtual issue order (which drives the 8-lane
            # DMA-semaphore round-robin) exactly matches ring order:
            # lane sharing then always pairs FIFO-ordered transfers and
            # a consumer's wait can never couple to a LATER unrelated
            # DMA (that cost C0 ~5us when v0 shared a lane with k1).
            # scalar: q0a q0b q0c q1a q1b q1c v1  (4.75 MiB)
            for i in range(3):
                t, lo, hi = qt[0][i]
                nc.scalar.dma_start(out=t, in_=q_ext[0, :, lo:hi, :])
            for i in range(3):
                t, lo, hi = qt[1][i]
                nc.scalar.dma_start(out=t, in_=q_ext[1, :, lo:hi, :])
            nc.scalar.dma_start(out=vt[1], in_=vt_ext[1])
            # sync: k0a k0b v0 k1a k1b q1d  (5.25 MiB) — v0 early so C0
            # can start right at bd0-ready
            for t, lo, hi in kt[0]:
                nc.sync.dma_start(out=t, in_=k_ext[0, :, lo:hi, :])
            nc.sync.dma_start(out=vt[0], in_=vt_ext[0])
            for t, lo, hi in kt[1]:
                nc.sync.dma_start(out=t, in_=k_ext[1, :, lo:hi, :])
            t, lo, hi = qt[1][3]
            nc.sync.dma_start(out=t, in_=q_ext[1, :, lo:hi, :])

            ident = singles.tile([_C, _C], f32)
            make_identity(nc, ident)
            nc.vector.memset(warm, 0.0)
            nc.vector.memset(bd[0], 0.0)
            nc.vector.memset(bd[1], 0.0)

        def emit_phase_a(b):
            gram = ps_sc.tile([2 * _C, 2, _C], f32, tag="gram")
            for j in range(_PAIRS):
                q_t, qlo, _ = next(x for x in qt[b] if x[1] <= j < x[2])
                k_t, klo, _ = next(x for x in kt[b] if x[1] <= j < x[2])
                nc.tensor.matmul(
                    gram,
                    lhsT=q_t[:, j - qlo, :],
                    rhs=k_t[:, j - klo, :],
                    start=(j == 0),
                    stop=(j == _PAIRS - 1),
                )
            return gram

        def emit_dummies():
            # PE warm-keepers bridging a softmax ACT/DVE window so HAM
            # never sees a long PE idle gap (=> K=4/8 for the next phase)
            wps = ps_w.tile([8, 8], f32, tag="wps")
            for _ in range(_N_DUMMY):
                nc.tensor.matmul(wps, lhsT=warm[:, 0:8], rhs=warm, start=True, stop=True)

        def emit_softmax(gram):
            # scores = even-w block + odd-w block of the pair Gram tile
            # (one operand staged to SBUF: one PSUM read per instruction)
            s0 = sm.tile([_C, _C], f32, tag="s0")
            nc.vector.tensor_copy(out=s0, in_=gram[0:_C, 0, :])
            scores = sm.tile([_C, _C], f32, tag="scores")
            nc.vector.tensor_tensor(
                out=scores,
                in0=gram[_C : 2 * _C, 1, :],
                in1=s0,
                op=mybir.AluOpType.add,
            )
            negmax = sm.tile([_C, 1], f32, tag="negmax")
            nc.vector.tensor_reduce(
                out=negmax,
                in_=scores,
                axis=mybir.AxisListType.X,
                op=mybir.AluOpType.max,
                negate=True,
            )
            e = sm.tile([_C, _C], f32, tag="e")
            ssum = sm.tile([_C, 1], f32, tag="ssum")
            nc.scalar.activation(
                out=e,
                in_=scores,
                func=mybir.ActivationFunctionType.Exp,
                bias=negmax,
                scale=1.0,
                accum_out=ssum,
            )
            rsum = sm.tile([_C, 1], f32, tag="rsum")
            nc.vector.reciprocal(out=rsum, in_=ssum)
            attn = sm.tile([_C, _C], f32, tag="attn")
            nc.vector.tensor_scalar_mul(attn, e, rsum)
            return attn

        def emit_bd(attn, b):
            # PE transpose, then block-diag bd with the 1/V_SCALE folded
            # into the two scale-casts (DVE + ACT in parallel)
            attnT_ps = ps_at.tile([_C, _C], f32, tag="attnT_ps")
            nc.tensor.transpose(attnT_ps, attn, ident)
            nc.vector.tensor_scalar_mul(
                bd[b][0:_C, 0, :], attnT_ps, float(1.0 / _V_SCALE)
            )
            nc.scalar.activation(
                out=bd[b][_C : 2 * _C, 1, :],
                in_=attnT_ps,
                func=mybir.ActivationFunctionType.Copy,
                scale=float(1.0 / _V_SCALE),
            )

        def emit_c_chunks(b, otile, wbase, p_lo, p_hi, par):
            """matmul+copy chunks for pairs [p_lo, p_hi) of batch b into
            otile starting at w offset (p_lo*2 - wbase)."""
            for wg in range(p_lo, p_hi, 4):  # 4 pairs per PSUM bank
                o_ps = ps_o.tile([_H, 8, _C], f32, tag="o_ps")
                for half in range(4):
                    nc.tensor.matmul(
                        o_ps[:, 2 * half : 2 * half + 2, :],
                        lhsT=vt[b][:, wg + half, :],
                        rhs=bd[b],
                        start=True,
                        stop=True,
                    )
                w0 = wg * 2 - wbase
                dst = otile[:, w0 : w0 + 8, :]
                if (wg // 4 + par) % 2 == 0:
                    nc.vector.tensor_copy(out=dst, in_=o_ps)
                else:
                    nc.scalar.activation(
                        out=dst, in_=o_ps, func=mybir.ActivationFunctionType.Copy
                    )

        # ---- the pinned chain
        with tc.tile_wait_until(0.010):
            gram0 = emit_phase_a(0)
        with tc.tile_wait_until(0.030):
            attn0 = emit_softmax(gram0)
            emit_dummies()
            emit_bd(attn0, 0)
        with tc.tile_wait_until(0.040):
            # C0: whole batch-0 output into SBUF; halves stored behind
            # the loads in each HWDGE ring's FIFO (SWDGE stores crawl
            # when issued while the HW rings still stream loads).
            emit_c_chunks(0, out0, 0, 0, 32, 0)
            nc.gpsimd.dma_start(out=o_ext[0, :, 0:64, :], in_=out0[:, 0:64, :])
        with tc.tile_wait_until(0.045):
            emit_c_chunks(0, out0, 0, 32, 64, 0)
            nc.scalar.dma_start(out=o_ext[0, :, 64:128, :], in_=out0[:, 64:128, :])
        with tc.tile_wait_until(0.060):
            gram1 = emit_phase_a(1)
        with tc.tile_wait_until(0.070):
            attn1 = emit_softmax(gram1)
            emit_dummies()
            emit_bd(attn1, 1)
        with tc.tile_wait_until(0.090):
            # C1: produce + store pieces as they complete
            store_rings = [
                [nc.sync],
                [nc.scalar],
                [nc.sync],
                [nc.sync, nc.scalar],
            ]
            for pc in range(_NSP):
                otile = op.tile([_H, _SPIECE, _C], bf16, tag="otile")
                emit_c_chunks(
                    1, otile, pc * _SPIECE, pc * (_SPIECE // 2), (pc + 1) * (_SPIECE // 2), pc
                )
                w0 = pc * _SPIECE
                rings = store_rings[pc]
                wstep = _SPIECE // len(rings)
                for ri, eng in enumerate(rings):
                    sl = slice(w0 + ri * wstep, w0 + (ri + 1) * wstep)
                    eng.dma_start(
                        out=o_ext[1, :, sl, :],
                        in_=otile[:, ri * wstep : (ri + 1) * wstep, :],
                    )

    nc.finalize()
    return nc


def _get_nc():
    if "nc" not in _cache:
        _cache["nc"] = _build_nc()
    return _cache["nc"]


def _prep_inputs(q, k, v):
    """Host-side layout prep: fp16 casts of Q/K, V scaled into fp8e3m4
    and transposed per w-pair."""
    q16 = q.astype(np.float16).reshape(_B, _H, _PAIRS, 2 * _C)
    k16 = k.astype(np.float16).reshape(_B, _H, _PAIRS, 2 * _C)
    v8 = (v * _V_SCALE).astype(ml_dtypes.float8_e3m4)  # [B, H, W, C]
    # vt[b, (dw c), j, h] = v[b, h, 2j+dw, c]
    x = v8.transpose(0, 2, 3, 1)  # [B, W, C, H]
    x = x.reshape(_B, _PAIRS, 2, _C, _H)  # [B, j, dw, C, H]
    vt = np.ascontiguousarray(x.transpose(0, 2, 3, 1, 4)).reshape(
        _B, 2 * _C, _PAIRS, _H
    )
    return q16, k16, vt


def run(inputs, trace=False):
    """Run the SPMD kernel. Returns (full_output, BassKernelResults)."""
    from concourse.bass_utils import run_bass_kernel_spmd

    q = np.asarray(inputs["query"], dtype=np.float32)
    k = np.asarray(inputs["keys"], dtype=np.float32)
    v = np.asarray(inputs["values"], dtype=np.float32)
    assert q.shape == (_B, _H, _W, _C), q.shape

    q16, k16, vt = _prep_inputs(q, k, v)

    nc = _get_nc()
    in_maps = []
    for i in range(_NCORES):
        sl = slice(i * _BPC, (i + 1) * _BPC)
        in_maps.append({"q16": q16[sl], "k16": k16[sl], "v8t": vt[sl]})

    res = run_bass_kernel_spmd(
        nc, in_maps, core_ids=list(range(_NCORES)), trace=trace
    )
    out = np.concatenate(
        [res.results[i]["out"].astype(np.float32) for i in range(_NCORES)], axis=0
    )
    return out, res


def kernel(**inputs) -> np.ndarray:
    out, _ = run(inputs, trace=False)
    return out
